# revision 18
# baseline (speedup 1.0000x reference)
"""DMoLE Linear (base W + masked multi-expert LoRA) on 8 Trainium2 NeuronCores.

Strategy (per sharding hint): data-parallel shard x over the 8192 flattened
tokens (1024 tokens/core); replicate W, b, and the tiny rank-16 LoRA tensors.
Each core computes a disjoint token-slice of the output, so no collectives.

Math per core (T=1024 tokens, D=2048, O=2048, E*R=128):
    y = x @ W^T + b + (x @ A_all^T * mask) @ B_all^T          (SCALING = 1.0)
The per-expert sum collapses: concatenating the E experts along the rank axis
gives A_all [E*R, D], B_all [O, E*R]; the LoRA delta is one extra K=128 step
accumulated into the same PSUM group as the 16 K=128 steps of the base matmul.

The PE contracts along the partition axis, so both matmul operands need
d-major layouts. Replicated weights (W, A, B) are laid out d-major on the
host (pure input marshaling, like the replication itself); the activation x
is transposed on-chip via PE identity transposes. All matmul operands are
float32r (1 cycle/row at moving dim 512, vs 4 cycles for plain fp32).

Engine plan: Sync issues all input DMAs (so prefetch never queues behind
compute-gated stores), Scalar issues output DMAs, DVE does PSUM eviction
(x^T casts, masked z eviction, bias-add on y), GPSIMD broadcasts the bias.
"""

import os
import numpy as np

B, S, D, O, E, R = 4, 2048, 2048, 2048, 8, 16
ER = E * R                      # 128
NCORES = 8
TOK = B * S                     # 8192
T = TOK // NCORES               # 1024 tokens per core
P = 128
NOC = 8                         # o-chunks of 256, processed in pairs
OC = O // NOC                   # 256
KD = D // P                     # 16 k-tiles

_CACHE = {}

# Set by kernel() when KERNEL_TRACE=1: (exec_time_ns, mean_exec_time_ns, tmpdir)
LAST_TIMING = None


def _build():
    from contextlib import ExitStack
    import concourse.tile as tile
    from concourse import bacc, mybir

    F32 = mybir.dt.float32
    F32R = mybir.dt.float32r

    nc = bacc.Bacc("TRN2", target_bir_lowering=False, debug=False)

    x_d = nc.dram_tensor("x", [T, D], F32R, kind="ExternalInput").ap()
    wt_d = nc.dram_tensor("wt", [D, O], F32R, kind="ExternalInput").ap()   # W^T
    at_d = nc.dram_tensor("at", [D, ER], F32R, kind="ExternalInput").ap()  # A_all^T
    bt_d = nc.dram_tensor("bt", [ER, O], F32R, kind="ExternalInput").ap()  # B_all^T
    bias_d = nc.dram_tensor("bias", [1, O], F32, kind="ExternalInput").ap()
    mask_d = nc.dram_tensor("mask", [ER, 1], F32, kind="ExternalInput").ap()
    id_d = nc.dram_tensor("ident", [P, P], F32R, kind="ExternalInput").ap()
    y_d = nc.dram_tensor("y", [T, O], F32, kind="ExternalOutput").ap()

    with tile.TileContext(nc) as tc, ExitStack() as ctx:
        const = ctx.enter_context(tc.tile_pool(name="const", bufs=1))
        big = ctx.enter_context(tc.tile_pool(name="big", bufs=1))
        wt_pool = ctx.enter_context(tc.tile_pool(name="wt", bufs=4))
        xstage = ctx.enter_context(tc.tile_pool(name="xstage", bufs=6))
        outp = ctx.enter_context(tc.tile_pool(name="outp", bufs=6))
        ps_tr = ctx.enter_context(tc.tile_pool(name="ps_tr", bufs=3, space="PSUM"))
        ps_y = ctx.enter_context(tc.tile_pool(name="ps_y", bufs=4, space="PSUM"))
        ps_z = ctx.enter_context(tc.tile_pool(name="ps_z", bufs=1, space="PSUM"))

        # The identity and the first x half-blocks gate the PE's first work —
        # they own the head of the Sync queue. All small consts go on the
        # Scalar DMA queue so they never head-block x.
        ident = const.tile([P, P], F32R)
        nc.sync.dma_start(out=ident[:], in_=id_d[:])

        mask_sb = const.tile([ER, 1], F32)
        nc.scalar.dma_start(out=mask_sb[:], in_=mask_d[:])
        bias_row = const.tile([1, O], F32)
        nc.scalar.dma_start(out=bias_row[:], in_=bias_d[:])
        # bias_bc is broadcast later (after the first z group) — the GPSIMD
        # custom op locks the SBUF port it shares with the DVE, which must
        # not happen while the DVE drains the early transpose casts.
        bias_bc = const.tile([P, O], F32)
        at_sb = const.tile([P, KD * ER], F32R)  # [d-in-tile, (d_i, er)]
        nc.scalar.dma_start(
            out=at_sb[:].rearrange("p (i c) -> p i c", c=ER),
            in_=at_d.rearrange("(i p) c -> p i c", p=P),
        )
        bt_sb = const.tile([ER, O], F32R)
        nc.scalar.dma_start(out=bt_sb[:], in_=bt_d[:])

        # xT[:, d_i*T + t] = x[t, d_i*128 + p]; zT[er, t] = masked z
        xT = big.tile([P, KD * T], F32R)
        zT = big.tile([ER, T], F32R)

        HD = D // 2
        wt_tiles = {}

        def load_wt(oc):
            wt = wt_pool.tile([P, KD * OC], F32R, tag="wt")  # [d, (d_i, o)]
            wt_tiles[oc] = wt
            return wt

        def load_wt_slices(oc, d_lo, d_hi):
            wt = wt_tiles[oc]
            for d_i in range(d_lo, d_hi):
                nc.sync.dma_start(
                    out=wt[:, d_i * OC:(d_i + 1) * OC],
                    in_=wt_d[d_i * P:(d_i + 1) * P, oc * OC:(oc + 1) * OC],
                )

        def xpose_pair(tg, p, quarters=False):
            """Transpose one t-block pair (256 tokens); each PSUM eviction
            cast is [128, 256]. `quarters` splits the x loads 4-ways so the
            very first transposes start as early as possible."""
            tA = tg * 512 + 2 * p * P
            for h in range(2):
                nq = 4 if quarters and h == 0 else 1
                QW = HD // nq
                parts = []
                for row0 in (tA, tA + P):
                    row_parts = []
                    for q in range(nq):
                        xs = xstage.tile([P, QW], F32R, tag=f"xs{nq}")
                        nc.sync.dma_start(
                            out=xs[:],
                            in_=x_d[row0:row0 + P,
                                    h * HD + q * QW:h * HD + (q + 1) * QW],
                        )
                        row_parts.append(xs)
                    parts.append(row_parts)
                if tg == 0 and p == 0 and h == 1:
                    load_wt_slices(0, 0, KD)
                if tg == 0 and p == 1 and h == 0:
                    load_wt_slices(1, 0, KD)
                for dj in range(KD // 2):
                    d_i = h * (KD // 2) + dj
                    qi, qo = divmod(dj * P, QW)
                    pt = ps_tr.tile([P, 2 * P], F32R, tag="pt")
                    nc.tensor.matmul(
                        pt[:, 0:P], parts[0][qi][:, qo:qo + P], ident[:],
                        is_transpose=True,
                    )
                    nc.tensor.matmul(
                        pt[:, P:2 * P], parts[1][qi][:, qo:qo + P], ident[:],
                        is_transpose=True,
                    )
                    nc.vector.tensor_copy(
                        xT[:, d_i * T + tA:d_i * T + tA + 2 * P], pt[:]
                    )

        def z_group(tg):
            zp = ps_z.tile([ER, 512], mybir.dt.float32, tag="zp")
            for d_i in range(KD):
                nc.tensor.matmul(
                    zp[:],
                    at_sb[:, d_i * ER:(d_i + 1) * ER],
                    xT[:, d_i * T + tg * 512:d_i * T + (tg + 1) * 512],
                    start=(d_i == 0),
                    stop=(d_i == KD - 1),
                )
            # mask + round to f32r while evicting PSUM
            nc.vector.tensor_scalar_mul(
                zT[:, tg * 512:(tg + 1) * 512], zp[:], mask_sb[:]
            )

        def base_open2(oc, tb):
            """Open two accumulation groups over the chunk pair (oc, oc+1);
            each stationary x^T load feeds both chunks' matmuls."""
            wtE, wtO = wt_tiles[oc], wt_tiles[oc + 1]
            ypE = ps_y.tile([P, OC], mybir.dt.float32, tag="yp")
            ypO = ps_y.tile([P, OC], mybir.dt.float32, tag="yp")
            for d_i in range(KD):
                xw = xT[:, d_i * T + tb * P:d_i * T + (tb + 1) * P]
                nc.tensor.matmul(
                    ypE[:], xw, wtE[:, d_i * OC:(d_i + 1) * OC],
                    start=(d_i == 0), stop=False,
                )
                nc.tensor.matmul(
                    ypO[:], xw, wtO[:, d_i * OC:(d_i + 1) * OC],
                    start=(d_i == 0), stop=False,
                )
            return ypE, ypO

        def finish2(oc, tb, yps):
            zw = zT[:, tb * P:(tb + 1) * P]
            for j, yp in enumerate(yps):
                c = oc + j
                nc.tensor.matmul(
                    yp[:], zw, bt_sb[:, c * OC:(c + 1) * OC],
                    start=False, stop=True,
                )
            for j, yp in enumerate(yps):
                c = oc + j
                ot = outp.tile([P, OC], F32, tag="ot")
                nc.vector.tensor_add(ot[:], yp[:], bias_bc[:, c * OC:(c + 1) * OC])
                nc.scalar.dma_start(
                    out=y_d[tb * P:(tb + 1) * P, c * OC:(c + 1) * OC],
                    in_=ot[:],
                )

        def mains2(oc, tb_lo, tb_hi):
            for tb in range(tb_lo, tb_hi):
                finish2(oc, tb, base_open2(oc, tb))

        # Per 512-token group: both transpose pairs first (PE transposes must
        # NEVER interleave inside an open accumulation group — transpose-mode
        # matmuls corrupt other banks' accumulation state). Then open the
        # first base pair-group (its early K-steps only need pair-0 casts,
        # so the PE streams while the DVE drains pair-1 casts), z, finish.
        load_wt(0)
        load_wt(1)
        for tg in range(2):
            tb0 = tg * 4
            xpose_pair(tg, 0, quarters=(tg == 0))
            xpose_pair(tg, 1)
            yps = base_open2(0, tb0)
            z_group(tg)
            if tg == 0:
                nc.gpsimd.partition_broadcast(bias_bc[:], bias_row[:])
            finish2(0, tb0, yps)
            mains2(0, tb0 + 1, tb0 + 4)
        for oc in range(2, NOC, 2):
            load_wt(oc)
            load_wt_slices(oc, 0, KD)
            load_wt(oc + 1)
            load_wt_slices(oc + 1, 0, KD)
            mains2(oc, 0, T // P)

    nc.compile()
    return nc


def _get_nc():
    if "nc" not in _CACHE:
        _CACHE["nc"] = _build()
    return _CACHE["nc"]


def kernel(x, W, b, lora_A, lora_B, expert_mask):
    global LAST_TIMING
    from concourse.bass_utils import run_bass_kernel_spmd

    nc = _get_nc()

    xf = np.ascontiguousarray(x.reshape(TOK, D), dtype=np.float32)
    wt = np.ascontiguousarray(np.asarray(W, dtype=np.float32).T)  # [D, O]
    at = np.ascontiguousarray(
        np.transpose(np.asarray(lora_A, dtype=np.float32), (2, 0, 1)).reshape(D, ER)
    )
    bt = np.ascontiguousarray(
        np.transpose(np.asarray(lora_B, dtype=np.float32), (0, 2, 1)).reshape(ER, O)
    )
    bias = np.ascontiguousarray(b.reshape(1, O), dtype=np.float32)
    mask = np.repeat(np.asarray(expert_mask).astype(np.float32), R).reshape(ER, 1)
    mask = np.ascontiguousarray(mask)
    ident = np.eye(P, dtype=np.float32)

    shared = {"wt": wt, "at": at, "bt": bt, "bias": bias, "mask": mask, "ident": ident}
    in_maps = [
        {"x": xf[i * T:(i + 1) * T], **shared} for i in range(NCORES)
    ]

    trace = os.environ.get("KERNEL_TRACE", "0") == "1"
    kw = {}
    if trace:
        import sys
        import types
        import tempfile

        if "antenv.axon_hooks" not in sys.modules:
            import trn_agent_boot.trn_boot as tb

            hook = tb._ntff_profile_via_ctypes("/opt/axon/libaxon_pjrt.so")
            mod = types.ModuleType("antenv.axon_hooks")
            mod.get_axon_ntff_profile_hook = lambda: hook
            sys.modules["antenv.axon_hooks"] = mod
        kw = {"trace": True, "tmpdir": tempfile.mkdtemp(prefix="dmole_trace_")}

    res = run_bass_kernel_spmd(nc, in_maps, list(range(NCORES)), **kw)
    if trace:
        LAST_TIMING = (res.exec_time_ns, res.mean_exec_time_ns, kw.get("tmpdir"))

    y = np.concatenate([res.results[i]["y"] for i in range(NCORES)], axis=0)
    return np.ascontiguousarray(y.reshape(B, S, O), dtype=np.float32)


# revision 22
# speedup vs baseline: 1.1279x; 1.1279x over previous
"""DMoLE Linear (base W + masked multi-expert LoRA) on 8 Trainium2 NeuronCores.

Strategy (per sharding hint): data-parallel shard x over the 8192 flattened
tokens (1024 tokens/core); replicate W, b, and the tiny rank-16 LoRA tensors.
Each core computes a disjoint token-slice of the output, so no collectives.

Math per core (T=1024 tokens, D=2048, O=2048, E*R=128):
    y = x @ W^T + b + (x @ A_all^T * mask) @ B_all^T          (SCALING = 1.0)
The per-expert sum collapses: concatenating the E experts along the rank axis
gives A_all [E*R, D], B_all [O, E*R]; the LoRA delta is one extra K=128 step
accumulated into the same PSUM group as the 16 K=128 steps of the base matmul.

The PE contracts along the partition axis, so both matmul operands need
d-major layouts. Replicated weights (W, A, B) are laid out d-major on the
host (pure input marshaling, like the replication itself); the activation x
is transposed on-chip via PE identity transposes. All matmul operands are
float32r (1 cycle/row at moving dim 512, vs 4 cycles for plain fp32).

Engine plan: Sync issues all input DMAs (so prefetch never queues behind
compute-gated stores), Scalar issues output DMAs, DVE does PSUM eviction
(x^T casts, masked z eviction, bias-add on y), GPSIMD broadcasts the bias.
"""

import os
import numpy as np

B, S, D, O, E, R = 4, 2048, 2048, 2048, 8, 16
ER = E * R                      # 128
NCORES = 8
TOK = B * S                     # 8192
T = TOK // NCORES               # 1024 tokens per core
P = 128
NOC = 4                         # o-chunks of 512
OC = O // NOC                   # 512
KD = D // P                     # 16 k-tiles

_CACHE = {}

# Set by kernel() when KERNEL_TRACE=1: (exec_time_ns, mean_exec_time_ns, tmpdir)
LAST_TIMING = None


def _build():
    from contextlib import ExitStack
    import concourse.tile as tile
    from concourse import bacc, mybir

    F32 = mybir.dt.float32
    F32R = mybir.dt.float32r

    nc = bacc.Bacc("TRN2", target_bir_lowering=False, debug=False)

    x_d = nc.dram_tensor("x", [T, D], F32R, kind="ExternalInput").ap()
    wt_d = nc.dram_tensor("wt", [D, O], F32R, kind="ExternalInput").ap()   # W^T
    at_d = nc.dram_tensor("at", [D, ER], F32R, kind="ExternalInput").ap()  # A_all^T
    bt_d = nc.dram_tensor("bt", [ER, O], F32R, kind="ExternalInput").ap()  # B_all^T
    bias_d = nc.dram_tensor("bias", [1, O], F32, kind="ExternalInput").ap()
    mask_d = nc.dram_tensor("mask", [ER, 1], F32, kind="ExternalInput").ap()
    id_d = nc.dram_tensor("ident", [P, P], F32R, kind="ExternalInput").ap()
    y_d = nc.dram_tensor("y", [T, O], F32, kind="ExternalOutput").ap()

    with tile.TileContext(nc) as tc, ExitStack() as ctx:
        const = ctx.enter_context(tc.tile_pool(name="const", bufs=1))
        big = ctx.enter_context(tc.tile_pool(name="big", bufs=1))
        wt_pool = ctx.enter_context(tc.tile_pool(name="wt", bufs=2))
        xstage = ctx.enter_context(tc.tile_pool(name="xstage", bufs=16))
        outp = ctx.enter_context(tc.tile_pool(name="outp", bufs=4))
        ps_tr = ctx.enter_context(tc.tile_pool(name="ps_tr", bufs=4, space="PSUM"))
        ps_y = ctx.enter_context(tc.tile_pool(name="ps_y", bufs=3, space="PSUM"))
        ps_z = ctx.enter_context(tc.tile_pool(name="ps_z", bufs=1, space="PSUM"))

        # The identity and the first x half-blocks gate the PE's first work —
        # they own the head of the Sync queue. All small consts go on the
        # Scalar DMA queue so they never head-block x.
        ident = const.tile([P, P], F32R)
        nc.sync.dma_start(out=ident[:], in_=id_d[:])

        mask_sb = const.tile([ER, 1], F32)
        nc.scalar.dma_start(out=mask_sb[:], in_=mask_d[:])
        bias_row = const.tile([1, O], F32)
        nc.scalar.dma_start(out=bias_row[:], in_=bias_d[:])
        bias_bc = const.tile([P, O], F32)
        nc.gpsimd.partition_broadcast(bias_bc[:], bias_row[:])
        at_sb = const.tile([P, KD * ER], F32R)  # [d-in-tile, (d_i, er)]
        nc.scalar.dma_start(
            out=at_sb[:].rearrange("p (i c) -> p i c", c=ER),
            in_=at_d.rearrange("(i p) c -> p i c", p=P),
        )
        bt_sb = const.tile([ER, O], F32R)
        nc.scalar.dma_start(out=bt_sb[:], in_=bt_d[:])

        # xT[:, d_i*T + t] = x[t, d_i*128 + p]; zT[er, t] = masked z
        xT = big.tile([P, KD * T], F32R)
        zT = big.tile([ER, T], F32R)

        HD = D // 2
        wt_tiles = {}

        def load_wt(oc):
            wt = wt_pool.tile([P, KD * OC], F32R, tag="wt")  # [d, (d_i, o)]
            wt_tiles[oc] = wt
            return wt

        def load_wt_slices(oc, d_lo, d_hi):
            wt = wt_tiles[oc]
            for d_i in range(d_lo, d_hi):
                nc.sync.dma_start(
                    out=wt[:, d_i * OC:(d_i + 1) * OC],
                    in_=wt_d[d_i * P:(d_i + 1) * P, oc * OC:(oc + 1) * OC],
                )

        QW = D // 4  # 512-float quarter-rows: 4 d-tiles per stage tile

        def xpose_quad(tg):
            """Transpose a whole 512-token group, 4 t-blocks at a time per
            d-tile, so each PSUM eviction is one [128, 512] op. Evictions
            alternate DVE / ACT so neither engine paces the PE. Casts land
            in d_i order, letting the following base groups' K-loops trail
            the eviction stream with fine-grained overlap."""
            tA = tg * 512
            for h in range(2):
                parts = {}
                for q in range(2):
                    for tb4 in range(4):
                        xs = xstage.tile([P, QW], F32R, tag="xs")
                        c0 = h * HD + q * QW
                        nc.sync.dma_start(
                            out=xs[:],
                            in_=x_d[tA + tb4 * P:tA + (tb4 + 1) * P, c0:c0 + QW],
                        )
                        parts[(q, tb4)] = xs
                    if tg == 0 and h == 0:
                        load_wt_slices(0, q * 8, q * 8 + 8)
                for dj in range(KD // 2):
                    d_i = h * (KD // 2) + dj
                    q, rem = divmod(dj, 4)
                    pt = ps_tr.tile([P, 4 * P], F32R, tag="pt")
                    for tb4 in range(4):
                        nc.tensor.matmul(
                            pt[:, tb4 * P:(tb4 + 1) * P],
                            parts[(q, tb4)][:, rem * P:(rem + 1) * P],
                            ident[:],
                            is_transpose=True,
                        )
                    dst = xT[:, d_i * T + tA:d_i * T + tA + 512]
                    if d_i % 2 == 0:
                        nc.vector.tensor_copy(dst, pt[:])
                    else:
                        nc.scalar.activation(
                            dst, pt[:], mybir.ActivationFunctionType.Copy
                        )

        def z_group(tg):
            zp = ps_z.tile([ER, 512], mybir.dt.float32, tag="zp")
            for d_i in range(KD):
                nc.tensor.matmul(
                    zp[:],
                    at_sb[:, d_i * ER:(d_i + 1) * ER],
                    xT[:, d_i * T + tg * 512:d_i * T + (tg + 1) * 512],
                    start=(d_i == 0),
                    stop=(d_i == KD - 1),
                )
            # mask + round to f32r while evicting PSUM
            nc.vector.tensor_scalar_mul(
                zT[:, tg * 512:(tg + 1) * 512], zp[:], mask_sb[:]
            )

        def base_open(oc, tb):
            wt = wt_tiles[oc]
            yp = ps_y.tile([P, OC], mybir.dt.float32, tag="yp")
            for d_i in range(KD):
                nc.tensor.matmul(
                    yp[:],
                    xT[:, d_i * T + tb * P:d_i * T + (tb + 1) * P],
                    wt[:, d_i * OC:(d_i + 1) * OC],
                    start=(d_i == 0),
                    stop=False,
                )
            return yp

        def finish(oc, tb, yp):
            nc.tensor.matmul(
                yp[:],
                zT[:, tb * P:(tb + 1) * P],
                bt_sb[:, oc * OC:(oc + 1) * OC],
                start=False,
                stop=True,
            )
            ot = outp.tile([P, OC], F32, tag="ot")
            nc.vector.tensor_add(ot[:], yp[:], bias_bc[:, oc * OC:(oc + 1) * OC])
            nc.scalar.dma_start(
                out=y_d[tb * P:(tb + 1) * P, oc * OC:(oc + 1) * OC],
                in_=ot[:],
            )

        def mains(oc, tb_lo, tb_hi):
            for tb in range(tb_lo, tb_hi):
                finish(oc, tb, base_open(oc, tb))

        # Per 512-token group: both transpose pairs first (PE transposes must
        # NEVER interleave inside an open accumulation group — transpose-mode
        # matmuls corrupt other banks' accumulation state). Then open the
        # first two base groups (their early K-steps only need pair-0 casts,
        # so the PE streams while the DVE drains pair-1 casts), z, finish.
        load_wt(0)
        for tg in range(2):
            tb0 = tg * 4
            xpose_quad(tg)
            ypA = base_open(0, tb0)
            ypB = base_open(0, tb0 + 1)
            z_group(tg)
            finish(0, tb0, ypA)
            finish(0, tb0 + 1, ypB)
            mains(0, tb0 + 2, tb0 + 4)
        for oc in range(1, NOC):
            load_wt(oc)
            load_wt_slices(oc, 0, KD)
            mains(oc, 0, T // P)

    nc.compile()
    return nc


def _get_nc():
    if "nc" not in _CACHE:
        _CACHE["nc"] = _build()
    return _CACHE["nc"]


def kernel(x, W, b, lora_A, lora_B, expert_mask):
    global LAST_TIMING
    from concourse.bass_utils import run_bass_kernel_spmd

    nc = _get_nc()

    xf = np.ascontiguousarray(x.reshape(TOK, D), dtype=np.float32)
    wt = np.ascontiguousarray(np.asarray(W, dtype=np.float32).T)  # [D, O]
    at = np.ascontiguousarray(
        np.transpose(np.asarray(lora_A, dtype=np.float32), (2, 0, 1)).reshape(D, ER)
    )
    bt = np.ascontiguousarray(
        np.transpose(np.asarray(lora_B, dtype=np.float32), (0, 2, 1)).reshape(ER, O)
    )
    bias = np.ascontiguousarray(b.reshape(1, O), dtype=np.float32)
    mask = np.repeat(np.asarray(expert_mask).astype(np.float32), R).reshape(ER, 1)
    mask = np.ascontiguousarray(mask)
    ident = np.eye(P, dtype=np.float32)

    shared = {"wt": wt, "at": at, "bt": bt, "bias": bias, "mask": mask, "ident": ident}
    in_maps = [
        {"x": xf[i * T:(i + 1) * T], **shared} for i in range(NCORES)
    ]

    trace = os.environ.get("KERNEL_TRACE", "0") == "1"
    kw = {}
    if trace:
        import sys
        import types
        import tempfile

        if "antenv.axon_hooks" not in sys.modules:
            import trn_agent_boot.trn_boot as tb

            hook = tb._ntff_profile_via_ctypes("/opt/axon/libaxon_pjrt.so")
            mod = types.ModuleType("antenv.axon_hooks")
            mod.get_axon_ntff_profile_hook = lambda: hook
            sys.modules["antenv.axon_hooks"] = mod
        kw = {"trace": True, "tmpdir": tempfile.mkdtemp(prefix="dmole_trace_")}

    res = run_bass_kernel_spmd(nc, in_maps, list(range(NCORES)), **kw)
    if trace:
        LAST_TIMING = (res.exec_time_ns, res.mean_exec_time_ns, kw.get("tmpdir"))

    y = np.concatenate([res.results[i]["y"] for i in range(NCORES)], axis=0)
    return np.ascontiguousarray(y.reshape(B, S, O), dtype=np.float32)
